# revision 1
# baseline (speedup 1.0000x reference)
"""Trainium2 Bass kernel for expected-calibration-error (ECE) over [N,C] logits.

Contract: kernel(logits, targets) -> np.float32 scalar (shape ()), matching

    probs = softmax(logits); conf = max(probs); pred = argmax(probs)
    acc = (pred == targets); bins of width 1/10 over (k/10, (k+1)/10]
    ECE = sum_k |avg_conf_k - avg_acc_k| * count_k / N

Strategy (data-parallel over 8 NeuronCores, rows sharded):
  * Host packs the class index into the low 7 mantissa bits of each logit
    (value perturbation <= 1.5e-5 relative). A single segmented reduce_max
    per row then yields BOTH the max logit and, in its low mantissa bits,
    the argmax class — one DVE pass instead of two.
  * conf = exp(max) / sum_c exp(logit_c)  (no max-subtraction needed:
    |logits| < ~6 so exp() is safe in f32).  exp runs on the scalar
    (activation) engine; the 128-wide row sums are folded 128->64 on
    GpSimd, then segment-reduced on the vector engine.
  * Per-tile heavy passes only write per-row (max, sumexp) columns into
    group buffers; all the small per-row ops (conf, acc, bin masks) are
    batched once per GROUP of tiles so tiny cross-engine-dependent ops
    don't head-of-line-block the in-order engines.
  * Per-row bin membership is encoded as cumulative masks
    g[k] = (conf > k/10), k = 0..10, and the per-bin (count, sum_conf,
    sum_acc) triples are produced by the tensor engine: for each block of
    128 rows, matmul(lhsT=[ones|conf|acc] (128x3), rhs=g (128x11))
    accumulates [3,11] cumulative stats in PSUM across the whole shard.
  * Host sums the 8 [3,11] outputs, differences adjacent cumulative
    columns to get per-bin stats, and applies the final ECE formula.
"""

import numpy as np

# Problem constants (hardcoded per harness contract).
N = 262144
C = 128
P = 128          # SBUF partitions
NB = 10          # calibration bins
NCORES = 8
T = 32           # rows per partition per tile
GK = 4           # tiles per small-op group
ROWS_PER_CORE = N // NCORES          # 32768
NTILES = ROWS_PER_CORE // (P * T)    # 8

_CACHE = {}

# perf-tuning knobs used by kernel()
KNOBS = dict(gp_fold=True, gp_maxfold_tiles=0, act_accum_blocks=0)


def build(ntiles=NTILES, t_rows=T, gk=GK, gp_fold=True, gp_maxfold_tiles=0,
          act_accum_blocks=0):
    """Build the Bass module. Returns nc.

    gp_fold: fold the exp() row halves 128->64 on GpSimd before the DVE
        row-sum reduce (halves DVE reduce_sum work).
    gp_maxfold_tiles: for the first k tiles of every group, also fold the
        row max 128->64 on GpSimd before the DVE reduce_max (DVE/GpSimd
        load-balance knob).
    act_accum_blocks: for the first k 128-row blocks of every tile, compute
        the row sum-of-exp on the scalar engine via per-block activation
        accum_out (skips the GpSimd fold and DVE reduce for those blocks).
    """
    import concourse.bacc as bacc
    import concourse.tile as tile
    from concourse import mybir

    f32 = mybir.dt.float32
    i32 = mybir.dt.int32
    Alu = mybir.AluOpType
    Act = mybir.ActivationFunctionType
    X = mybir.AxisListType.X

    assert ntiles % gk == 0
    ngroups = ntiles // gk
    gw = gk * t_rows  # group width (row-blocks per group)
    ab = act_accum_blocks
    assert 0 <= ab <= t_rows
    rb = t_rows - ab  # blocks on the fold+reduce path

    nc = bacc.Bacc(trn_type="TRN2")

    y_d = nc.dram_tensor("y", [ntiles, P, t_rows * C], f32, kind="ExternalInput")
    # tcode grouped to allow one DMA per group: [ngroups, P, gw]
    tcode_d = nc.dram_tensor("tcode", [ngroups, P, gw], i32, kind="ExternalInput")
    thr_d = nc.dram_tensor("thr", [1, NB + 1], f32, kind="ExternalInput")
    out_d = nc.dram_tensor("gstats", [3, NB + 1], f32, kind="ExternalOutput")

    with tile.TileContext(nc) as tc:
        with (
            tc.tile_pool(name="io", bufs=4) as io_pool,
            tc.tile_pool(name="ep", bufs=2) as e_pool,
            tc.tile_pool(name="fp", bufs=2) as f_pool,
            tc.tile_pool(name="grp", bufs=2) as grp_pool,
            tc.tile_pool(name="single", bufs=1) as single,
            tc.tile_pool(name="psum", bufs=1, space="PSUM") as psum_pool,
        ):
            thr_sb = single.tile([P, NB + 1], f32)
            nc.sync.dma_start(out=thr_sb[:], in_=thr_d[:].partition_broadcast(P))
            c127 = single.tile([P, 1], i32)
            nc.vector.memset(c127[:], 127)

            pstats = psum_pool.tile([3, NB + 1], f32)

            for grp in range(ngroups):
                # --- phase 1: heavy per-tile passes -> group stat columns ---
                my_g = grp_pool.tile([P, gw], f32)
                s_g = grp_pool.tile([P, gw], f32)
                tc_g = grp_pool.tile([P, gw], i32)
                nc.sync.dma_start(out=tc_g[:], in_=tcode_d[grp])

                for ti in range(gk):
                    t = grp * gk + ti
                    o0, o1 = ti * t_rows, (ti + 1) * t_rows

                    y_t = io_pool.tile([P, t_rows * C], f32)
                    nc.sync.dma_start(out=y_t[:], in_=y_d[t])
                    y3 = y_t[:].rearrange("p (t c) -> p t c", c=C)

                    # row sums of exp(y):
                    #  - first `ab` blocks: scalar-engine exp with accum_out
                    #  - rest: big exp, GpSimd 128->64 fold, DVE reduce
                    if ab > 0:
                        Es = f_pool.tile([P, C], f32, name="Escratch")
                        for b in range(ab):
                            nc.scalar.activation(
                                out=Es[:], in_=y3[:, b, :], func=Act.Exp,
                                accum_out=s_g[:, o0 + b : o0 + b + 1],
                            )
                    if rb > 0:
                        E = e_pool.tile([P, rb * C], f32)
                        nc.scalar.activation(
                            out=E[:], in_=y_t[:, ab * C :], func=Act.Exp
                        )
                        E3 = E[:].rearrange("p (t c) -> p t c", c=C)

                        if gp_fold:
                            F = f_pool.tile([P, rb * (C // 2)], f32)
                            F3 = F[:].rearrange("p (t c) -> p t c", c=C // 2)
                            nc.gpsimd.tensor_tensor(
                                out=F3, in0=E3[:, :, 0 : C // 2],
                                in1=E3[:, :, C // 2 : C], op=Alu.add,
                            )
                            nc.vector.tensor_reduce(
                                out=s_g[:, o0 + ab : o1], in_=F3, axis=X, op=Alu.add
                            )
                        else:
                            nc.vector.tensor_reduce(
                                out=s_g[:, o0 + ab : o1], in_=E3, axis=X, op=Alu.add
                            )

                    # packed row max (value + argmax in low mantissa bits)
                    if ti < gp_maxfold_tiles:
                        M = f_pool.tile([P, t_rows * (C // 2)], f32, name="Mfold")
                        M3 = M[:].rearrange("p (t c) -> p t c", c=C // 2)
                        nc.gpsimd.tensor_tensor(
                            out=M3, in0=y3[:, :, 0 : C // 2],
                            in1=y3[:, :, C // 2 : C], op=Alu.max,
                        )
                        nc.vector.tensor_reduce(
                            out=my_g[:, o0:o1], in_=M3, axis=X, op=Alu.max
                        )
                    else:
                        nc.vector.tensor_reduce(
                            out=my_g[:, o0:o1], in_=y3, axis=X, op=Alu.max
                        )

                # --- phase 2: batched small ops over the whole group ---
                maxE = grp_pool.tile([P, gw], f32)
                nc.scalar.activation(out=maxE[:], in_=my_g[:], func=Act.Exp)
                rs = grp_pool.tile([P, gw], f32)
                nc.vector.reciprocal(out=rs[:], in_=s_g[:])

                rhs3 = grp_pool.tile([P, 3, gw], f32)
                nc.gpsimd.memset(rhs3[:, 0, :], 1.0)
                nc.vector.tensor_tensor(
                    out=rhs3[:, 1, :], in0=maxE[:], in1=rs[:], op=Alu.mult
                )

                # acc: (packed-max mantissa & 127) == (127 - target)
                accx = grp_pool.tile([P, gw], i32)
                nc.vector.scalar_tensor_tensor(
                    out=accx[:], in0=my_g[:].bitcast(i32), scalar=c127[:],
                    in1=tc_g[:], op0=Alu.bitwise_and, op1=Alu.bitwise_xor,
                )
                nc.vector.tensor_scalar(
                    out=rhs3[:, 2, :], in0=accx[:], scalar1=0, scalar2=None,
                    op0=Alu.is_equal,
                )

                # cumulative bin masks g[k] = conf > k/10
                g = grp_pool.tile([P, gw, NB + 1], f32)
                conf_b = rhs3[:, 1, :].unsqueeze(2).broadcast_to([P, gw, NB + 1])
                thr_b = thr_sb[:].unsqueeze(1).broadcast_to([P, gw, NB + 1])
                nc.vector.tensor_tensor(out=g[:], in0=conf_b, in1=thr_b, op=Alu.is_gt)

                # per-128-row-block cumulative histogram triples on PE
                for j in range(gw):
                    nc.tensor.matmul(
                        pstats[:],
                        rhs3[:, :, j],
                        g[:, j, :],
                        start=(grp == 0 and j == 0),
                        stop=(grp == ngroups - 1 and j == gw - 1),
                        skip_group_check=True,
                    )

            stats_sb = single.tile([3, NB + 1], f32)
            nc.vector.tensor_copy(out=stats_sb[:], in_=pstats[:])
            nc.sync.dma_start(out=out_d[:], in_=stats_sb[:])

    nc.compile()
    return nc


def prep_inputs(logits, targets, ntiles=NTILES, t_rows=T, gk=GK, ncores=NCORES):
    """Pack + shard host inputs. Returns list of per-core in_maps."""
    l = np.ascontiguousarray(np.asarray(logits, dtype=np.float32))
    tg = np.asarray(targets).astype(np.int64)
    n = l.shape[0]

    yb = l.view(np.int32) & np.int32(~127)
    yb = yb | (127 - np.arange(C, dtype=np.int32))[None, :]
    y = yb.view(np.float32)

    tcode = (127 - tg).astype(np.int32)
    thr = (np.arange(NB + 1, dtype=np.float32) / NB).reshape(1, NB + 1)

    ngroups = ntiles // gk
    rpc = n // ncores
    in_maps = []
    for k in range(ncores):
        yk = y[k * rpc : (k + 1) * rpc].reshape(ntiles, P, t_rows * C)
        tk = (
            tcode[k * rpc : (k + 1) * rpc]
            .reshape(ngroups, gk, P, t_rows)
            .transpose(0, 2, 1, 3)
            .reshape(ngroups, P, gk * t_rows)
        )
        in_maps.append({"y": yk, "tcode": np.ascontiguousarray(tk), "thr": thr})
    return in_maps


def finalize(gstats_list, n=N):
    """Combine per-core cumulative [3, 11] stats into the ECE scalar."""
    G = np.zeros((3, NB + 1), dtype=np.float64)
    for gs in gstats_list:
        G += gs.astype(np.float64)
    per = G[:, 0:NB] - G[:, 1 : NB + 1]
    counts, sum_conf, sum_acc = per[0], per[1], per[2]
    safe = np.maximum(counts, 1.0)
    avg_conf = sum_conf / safe
    avg_acc = sum_acc / safe
    prop = counts / float(n)
    ece = np.where(counts > 0, np.abs(avg_conf - avg_acc) * prop, 0.0).sum()
    return np.array(ece, dtype=np.float32)


LAST_RESULTS = None  # BassKernelResults of the most recent kernel() call


def kernel(logits, targets):
    global LAST_RESULTS
    from concourse.bass_utils import run_bass_kernel_spmd

    key = (NTILES, T, GK, tuple(sorted(KNOBS.items())))
    if key not in _CACHE:
        _CACHE[key] = build(NTILES, T, GK, **KNOBS)
    nc = _CACHE[key]

    in_maps = prep_inputs(logits, targets)
    res = run_bass_kernel_spmd(nc, in_maps, core_ids=list(range(NCORES)))
    LAST_RESULTS = res
    return finalize([r["gstats"] for r in res.results])



# revision 4
# speedup vs baseline: 1.0266x; 1.0266x over previous
"""Trainium2 Bass kernel for expected-calibration-error (ECE) over [N,C] logits.

Contract: kernel(logits, targets) -> np.float32 scalar (shape ()), matching

    probs = softmax(logits); conf = max(probs); pred = argmax(probs)
    acc = (pred == targets); bins of width 1/10 over (k/10, (k+1)/10]
    ECE = sum_k |avg_conf_k - avg_acc_k| * count_k / N

Strategy (data-parallel over 8 NeuronCores, rows sharded, fp16 on device):
  * Host casts logits to fp16 (halves HBM traffic; validated 2.3e-4 final
    rel err vs the 2e-2 gate) and gathers tval[i] = y16[i, targets[i]].
    Since tval is an element of row i, acc == (tval == rowmax) exactly
    reproduces argmax accuracy up to fp16 ties (~12 rows in 262k).
  * Per row: max via pairwise fp16 fold tree (tensor_tensor max runs in
    the DVE 2x_1p mode at 0.5 cyc/elem; tensor_reduce has no fast mode),
    sum-of-exp via ACT-engine exp (fp16 out) + fp16 fold tree.
    The first (widest) fold levels of selected tiles run on GpSimd to
    balance DVE/GpSimd occupancy.
  * conf = exp(max) * recip(sumexp); cumulative bin masks
    g[k] = conf > k/10 as [P, 11, gw] (packed last dim keeps DVE 2x);
    per-128-row-block matmul(lhsT=[1|conf|acc], rhs=g) accumulates
    cumulative [3,11] stats in PSUM across the whole shard.
  * Host sums the 8 [3,11] outputs, differences adjacent cumulative
    columns, applies the final ECE formula.
"""

import numpy as np

# Problem constants (hardcoded per harness contract).
N = 262144
C = 128
P = 128          # SBUF partitions
NB = 10          # calibration bins
NCORES = 8
T = 32           # rows per partition per tile
GK = 4           # tiles per small-op group
ROWS_PER_CORE = N // NCORES          # 32768
NTILES = ROWS_PER_CORE // (P * T)    # 8
NGROUPS = NTILES // GK
GW = GK * T                          # group width (stat columns)

_CACHE = {}

# perf-tuning knobs: per-tile count of leading fold levels run on GpSimd
# (rest of each tree runs on DVE). keys: tile index in core (0..NTILES-1).
KNOBS = dict(
    # NOTE: Pool-engine tensor_tensor only implements add (no max), so the
    # max trees must stay on DVE; GpSimd offloads sum-tree leading folds.
    gp_max_l=(0, 0, 0, 0, 0, 0, 0, 0),   # max-tree leading levels on GpSimd
    gp_sum_l=(1, 1, 1, 1, 1, 1, 1, 0),   # sum-tree leading levels on GpSimd
    fold_to=8,                           # fold tree down to this width, then one reduce
)


def build(gp_max_l, gp_sum_l, fold_to=8):
    import concourse.bacc as bacc
    import concourse.tile as tile
    from concourse import mybir

    f16 = mybir.dt.float16
    f32 = mybir.dt.float32
    Alu = mybir.AluOpType
    Act = mybir.ActivationFunctionType
    X = mybir.AxisListType.X

    nc = bacc.Bacc(trn_type="TRN2")

    y_d = nc.dram_tensor("y", [NTILES, P, T * C], f16, kind="ExternalInput")
    tv_d = nc.dram_tensor("tv", [NGROUPS, P, GW], f16, kind="ExternalInput")
    thr_d = nc.dram_tensor("thr", [1, (NB + 1) * GW], f16, kind="ExternalInput")
    out_d = nc.dram_tensor("gstats", [3, NB + 1], f32, kind="ExternalOutput")

    with nc.allow_low_precision("fp16 ECE pipeline; 2.3e-4 final rel err"):
        with tile.TileContext(nc) as tc:
            with (
                tc.tile_pool(name="io", bufs=3) as io_pool,
                tc.tile_pool(name="ep", bufs=3) as e_pool,
                tc.tile_pool(name="fs", bufs=2) as fs_pool,
                tc.tile_pool(name="fm", bufs=2) as fm_pool,
                tc.tile_pool(name="grp", bufs=2) as grp_pool,
                tc.tile_pool(name="single", bufs=1) as single,
                tc.tile_pool(name="psum", bufs=1, space="PSUM") as psum_pool,
            ):
                thrg = single.tile([P, (NB + 1) * GW], f16)
                nc.sync.dma_start(out=thrg[:], in_=thr_d[:].partition_broadcast(P))
                thrg3 = thrg[:].rearrange("p (a b) -> p a b", b=GW)

                pstats = psum_pool.tile([3, NB + 1], f32)

                def emit_tree(tag, src3, out_col, op, gp_levels, pool):
                    """Fold [P, T, 128] -> out_col [P, T] via pairwise halving
                    (2x-mode tensor_tensor) + one final tensor_reduce."""
                    cur = src3
                    w = C
                    lvl = 0
                    while w > fold_to:
                        h = w // 2
                        nxt = pool.tile([P, T, h], f16, name=f"{tag}{h}")
                        eng = nc.gpsimd if lvl < gp_levels else nc.vector
                        eng.tensor_tensor(
                            out=nxt[:], in0=cur[:, :, 0:h], in1=cur[:, :, h:w], op=op
                        )
                        cur = nxt[:]
                        w = h
                        lvl += 1
                    nc.vector.tensor_reduce(out=out_col, in_=cur, axis=X, op=op)

                for grp in range(NGROUPS):
                    my_g = grp_pool.tile([P, GW], f16)
                    s_g = grp_pool.tile([P, GW], f16)
                    tv_g = grp_pool.tile([P, GW], f16)
                    nc.sync.dma_start(out=tv_g[:], in_=tv_d[grp])

                    for ti in range(GK):
                        t = grp * GK + ti
                        o0, o1 = ti * T, (ti + 1) * T

                        y_t = io_pool.tile([P, T * C], f16)
                        nc.sync.dma_start(out=y_t[:], in_=y_d[t])
                        y3 = y_t[:].rearrange("p (t c) -> p t c", c=C)

                        E = e_pool.tile([P, T * C], f16)
                        nc.scalar.activation(out=E[:], in_=y_t[:], func=Act.Exp)
                        E3 = E[:].rearrange("p (t c) -> p t c", c=C)

                        if gp_max_l[t] > 0:
                            # max L1 on GpSimd: enqueue sum tree first on DVE
                            emit_tree("s", E3, s_g[:, o0:o1], Alu.add,
                                      gp_sum_l[t], fs_pool)
                            emit_tree("m", y3, my_g[:, o0:o1], Alu.max,
                                      gp_max_l[t], fm_pool)
                        else:
                            # max tree depends only on the DMA; run it first
                            emit_tree("m", y3, my_g[:, o0:o1], Alu.max,
                                      gp_max_l[t], fm_pool)
                            emit_tree("s", E3, s_g[:, o0:o1], Alu.add,
                                      gp_sum_l[t], fs_pool)

                    # --- batched small per-row ops over the whole group ---
                    maxE = grp_pool.tile([P, GW], f16)
                    nc.scalar.activation(out=maxE[:], in_=my_g[:], func=Act.Exp)
                    rs = grp_pool.tile([P, GW], f16)
                    nc.vector.reciprocal(out=rs[:], in_=s_g[:])

                    rhs3 = grp_pool.tile([P, 3, GW], f16)
                    nc.gpsimd.memset(rhs3[:, 0, :], 1.0)
                    nc.vector.tensor_tensor(
                        out=rhs3[:, 1, :], in0=maxE[:], in1=rs[:], op=Alu.mult
                    )
                    # tval is an element of the row, so tval == max <=> argmax hit
                    nc.vector.tensor_tensor(
                        out=rhs3[:, 2, :], in0=tv_g[:], in1=my_g[:], op=Alu.is_equal
                    )

                    g = grp_pool.tile([P, NB + 1, GW], f16)
                    conf_b = rhs3[:, 1, :].unsqueeze(1).broadcast_to([P, NB + 1, GW])
                    nc.vector.tensor_tensor(
                        out=g[:], in0=conf_b, in1=thrg3, op=Alu.is_gt
                    )

                    for j in range(GW):
                        nc.tensor.matmul(
                            pstats[:],
                            rhs3[:, :, j],
                            g[:, :, j],
                            start=(grp == 0 and j == 0),
                            stop=(grp == NGROUPS - 1 and j == GW - 1),
                            skip_group_check=True,
                        )

                stats_sb = single.tile([3, NB + 1], f32)
                nc.vector.tensor_copy(out=stats_sb[:], in_=pstats[:])
                nc.sync.dma_start(out=out_d[:], in_=stats_sb[:])

    nc.compile()
    return nc


def prep_inputs(logits, targets):
    """Cast + shard host inputs. Returns list of per-core in_maps."""
    l = np.asarray(logits, dtype=np.float32)
    tg = np.asarray(targets).astype(np.int64)
    n = l.shape[0]

    y16 = l.astype(np.float16)
    tval = y16[np.arange(n), tg]

    thrv = (np.arange(NB + 1, dtype=np.float32) / NB).astype(np.float16)
    thrv[NB] = np.float16(65504)  # bin mask 10 must stay empty even if conf rounds >1
    thr_flat = np.ascontiguousarray(np.repeat(thrv, GW).reshape(1, (NB + 1) * GW))

    rpc = n // NCORES
    in_maps = []
    for k in range(NCORES):
        yk = y16[k * rpc : (k + 1) * rpc].reshape(NTILES, P, T * C)
        tvk = (
            tval[k * rpc : (k + 1) * rpc]
            .reshape(NGROUPS, GK, P, T)
            .transpose(0, 2, 1, 3)
            .reshape(NGROUPS, P, GW)
        )
        in_maps.append({"y": yk, "tv": np.ascontiguousarray(tvk), "thr": thr_flat})
    return in_maps


def finalize(gstats_list, n=N):
    """Combine per-core cumulative [3, 11] stats into the ECE scalar."""
    G = np.zeros((3, NB + 1), dtype=np.float64)
    for gs in gstats_list:
        G += gs.astype(np.float64)
    per = G[:, 0:NB] - G[:, 1 : NB + 1]
    counts, sum_conf, sum_acc = per[0], per[1], per[2]
    safe = np.maximum(counts, 1.0)
    avg_conf = sum_conf / safe
    avg_acc = sum_acc / safe
    prop = counts / float(n)
    ece = np.where(counts > 0, np.abs(avg_conf - avg_acc) * prop, 0.0).sum()
    return np.array(ece, dtype=np.float32)


LAST_RESULTS = None  # BassKernelResults of the most recent kernel() call


def kernel(logits, targets):
    global LAST_RESULTS
    from concourse.bass_utils import run_bass_kernel_spmd

    key = (tuple(KNOBS["gp_max_l"]), tuple(KNOBS["gp_sum_l"]), KNOBS["fold_to"])
    if key not in _CACHE:
        _CACHE[key] = build(KNOBS["gp_max_l"], KNOBS["gp_sum_l"], KNOBS["fold_to"])
    nc = _CACHE[key]

    in_maps = prep_inputs(logits, targets)
    res = run_bass_kernel_spmd(nc, in_maps, core_ids=list(range(NCORES)))
    LAST_RESULTS = res
    return finalize([r["gstats"] for r in res.results])


# revision 5
# speedup vs baseline: 1.0780x; 1.0500x over previous
"""Trainium2 Bass kernel for expected-calibration-error (ECE) over [N,C] logits.

Contract: kernel(logits, targets) -> np.float32 scalar (shape ()), matching

    probs = softmax(logits); conf = max(probs); pred = argmax(probs)
    acc = (pred == targets); bins of width 1/10 over (k/10, (k+1)/10]
    ECE = sum_k |avg_conf_k - avg_acc_k| * count_k / N

Strategy (data-parallel over 8 NeuronCores, rows sharded, bf16 on device):
  * Host casts logits to bf16 (halves HBM traffic; 2.6e-3 final rel err vs
    the 2e-2 gate) and gathers tval[i] = y16[i, targets[i]]. Since tval is
    an element of row i, acc == (tval == rowmax) reproduces argmax accuracy
    up to bf16 ties.
  * Rows live on partitions; each row's 128 classes are stored in host
    bit-reversed block order [b6,b5,b4,b3,t,c_low3] so every pairwise fold
    level of a reduction tree combines two CONTIGUOUS halves. DVE
    tensor_tensor measures 2 elem/cycle/lane on bf16 (fp16 add is
    emulated ~6x slower; tensor_reduce is 1 elem/cycle) — so per-row max
    and sum-of-exp run as 4 contiguous bf16 folds + one width-8 reduce.
  * exp on the ACT engine (bf16 in/out). For selected tiles the whole sum
    chain runs on GpSimd in fp16 (the Pool engine only implements add, and
    only fp16 is fast there) to offload the DVE.
  * conf = exp(max) * recip(sumexp); cumulative bin masks g[k] = conf>k/10
    as [P, 11, gw] bf16; per-128-row-block matmul(lhsT=[1|conf|acc], rhs=g)
    accumulates cumulative [3,11] stats in PSUM across the whole shard.
  * Host sums the 8 [3,11] outputs, differences adjacent cumulative
    columns, applies the final ECE formula.
"""

import numpy as np

# Problem constants (hardcoded per harness contract).
N = 262144
C = 128
P = 128          # SBUF partitions
NB = 10          # calibration bins
NCORES = 8
T = 32           # rows per partition per tile
GK = 4           # tiles per small-op group
ROWS_PER_CORE = N // NCORES          # 32768
NTILES = ROWS_PER_CORE // (P * T)    # 8
NGROUPS = NTILES // GK
GW = GK * T                          # group width (stat columns)
FREE = T * C                         # 4096 elems per partition per tile

_CACHE = {}

KNOBS = dict(
    gp_tiles=(1, 4, 6),  # tiles whose sum chain runs fully on GpSimd (fp16)
)


def build(gp_tiles=(1, 4, 6)):
    import concourse.bacc as bacc
    import concourse.tile as tile
    from concourse import mybir

    f16 = mybir.dt.float16
    bf16 = mybir.dt.bfloat16
    f32 = mybir.dt.float32
    Alu = mybir.AluOpType
    Act = mybir.ActivationFunctionType
    X = mybir.AxisListType.X

    nc = bacc.Bacc(trn_type="TRN2")

    y_d = nc.dram_tensor("y", [NTILES, P, FREE], bf16, kind="ExternalInput")
    tv_d = nc.dram_tensor("tv", [NGROUPS, P, GW], bf16, kind="ExternalInput")
    thr_d = nc.dram_tensor("thr", [1, (NB + 1) * GW], bf16, kind="ExternalInput")
    out_d = nc.dram_tensor("gstats", [3, NB + 1], f32, kind="ExternalOutput")

    with nc.allow_low_precision("bf16 ECE pipeline; 2.6e-3 final rel err"):
        with tile.TileContext(nc) as tc:
            with (
                tc.tile_pool(name="io", bufs=3) as io_pool,
                tc.tile_pool(name="eb", bufs=2) as eb_pool,
                tc.tile_pool(name="ef", bufs=2) as ef_pool,
                tc.tile_pool(name="fm", bufs=2) as fm_pool,
                tc.tile_pool(name="fs", bufs=2) as fs_pool,
                tc.tile_pool(name="fg", bufs=2) as fg_pool,
                tc.tile_pool(name="grp", bufs=2) as grp_pool,
                tc.tile_pool(name="single", bufs=1) as single,
                tc.tile_pool(name="psum", bufs=1, space="PSUM") as psum_pool,
            ):
                thrg = single.tile([P, (NB + 1) * GW], bf16)
                nc.sync.dma_start(out=thrg[:], in_=thr_d[:].partition_broadcast(P))
                thrg3 = thrg[:].rearrange("p (a b) -> p a b", b=GW)

                pstats = psum_pool.tile([3, NB + 1], f32)

                def tree(eng, src, out_col, op, pool, dt, tag):
                    """src [P, 4096] (bit-reversed layout) -> out_col [P, T].
                    Four contiguous pairwise folds + one width-8 reduce.
                    Folds run on `eng`; the final reduce always on DVE."""
                    cur, w = src, FREE
                    while w > T * 8:
                        h = w // 2
                        nxt = pool.tile([P, h], dt, name=f"{tag}{h}")
                        eng.tensor_tensor(
                            out=nxt[:], in0=cur[:, 0:h], in1=cur[:, h:w], op=op
                        )
                        cur, w = nxt[:], h
                    nc.vector.tensor_reduce(
                        out=out_col, in_=cur.rearrange("p (t c) -> p t c", c=8),
                        axis=X, op=op,
                    )

                for grp in range(NGROUPS):
                    my_g = grp_pool.tile([P, GW], bf16)
                    s_g = grp_pool.tile([P, GW], bf16)
                    tv_g = grp_pool.tile([P, GW], bf16)
                    nc.sync.dma_start(out=tv_g[:], in_=tv_d[grp])

                    for ti in range(GK):
                        t = grp * GK + ti
                        o0, o1 = ti * T, (ti + 1) * T

                        y_t = io_pool.tile([P, FREE], bf16)
                        nc.sync.dma_start(out=y_t[:], in_=y_d[t])

                        # per-row max of y (DVE folds; Pool has no max op)
                        tree(nc.vector, y_t[:], my_g[:, o0:o1], Alu.max,
                             fm_pool, bf16, "m")

                        # per-row sum of exp(y)
                        if t in gp_tiles:
                            E = ef_pool.tile([P, FREE], f16)
                            nc.scalar.activation(out=E[:], in_=y_t[:], func=Act.Exp)
                            tree(nc.gpsimd, E[:], s_g[:, o0:o1], Alu.add,
                                 fg_pool, f16, "g")
                        else:
                            E = eb_pool.tile([P, FREE], bf16)
                            nc.scalar.activation(out=E[:], in_=y_t[:], func=Act.Exp)
                            tree(nc.vector, E[:], s_g[:, o0:o1], Alu.add,
                                 fs_pool, bf16, "s")

                    # --- batched small per-row ops over the whole group ---
                    maxE = grp_pool.tile([P, GW], bf16)
                    nc.scalar.activation(out=maxE[:], in_=my_g[:], func=Act.Exp)
                    rs = grp_pool.tile([P, GW], bf16)
                    nc.vector.reciprocal(out=rs[:], in_=s_g[:])

                    rhs3 = grp_pool.tile([P, 3, GW], bf16)
                    nc.gpsimd.memset(rhs3[:, 0, :], 1.0)
                    nc.vector.tensor_tensor(
                        out=rhs3[:, 1, :], in0=maxE[:], in1=rs[:], op=Alu.mult
                    )
                    # tval is an element of the row, so tval == max <=> argmax hit
                    nc.vector.tensor_tensor(
                        out=rhs3[:, 2, :], in0=tv_g[:], in1=my_g[:], op=Alu.is_equal
                    )

                    g = grp_pool.tile([P, NB + 1, GW], bf16)
                    conf_b = rhs3[:, 1, :].unsqueeze(1).broadcast_to([P, NB + 1, GW])
                    nc.vector.tensor_tensor(
                        out=g[:], in0=conf_b, in1=thrg3, op=Alu.is_gt
                    )

                    for j in range(GW):
                        nc.tensor.matmul(
                            pstats[:],
                            rhs3[:, :, j],
                            g[:, :, j],
                            start=(grp == 0 and j == 0),
                            stop=(grp == NGROUPS - 1 and j == GW - 1),
                            skip_group_check=True,
                        )

                stats_sb = single.tile([3, NB + 1], f32)
                nc.vector.tensor_copy(out=stats_sb[:], in_=pstats[:])
                nc.sync.dma_start(out=out_d[:], in_=stats_sb[:])

    nc.compile()
    return nc


def prep_inputs(logits, targets):
    """Cast + shard + fold-layout host inputs. Returns per-core in_maps."""
    import ml_dtypes

    bf16 = ml_dtypes.bfloat16
    l = np.asarray(logits, dtype=np.float32)
    tg = np.asarray(targets).astype(np.int64)
    n = l.shape[0]

    y16 = l.astype(bf16)
    tval = y16[np.arange(n), tg]

    thrv = (np.arange(NB + 1, dtype=np.float32) / NB).astype(bf16)
    thrv[NB] = bf16(3e38)  # bin mask 10 must stay empty even if conf rounds >1
    thr_flat = np.ascontiguousarray(np.repeat(thrv, GW).reshape(1, (NB + 1) * GW))

    rpc = n // NCORES
    in_maps = []
    for k in range(NCORES):
        yk = y16[k * rpc : (k + 1) * rpc].reshape(NTILES, P, T, C)
        # bit-reversed block layout: [t, c=(b6 b5 b4 b3 c3)] -> [b6 b5 b4 b3 t c3]
        yk = (
            yk.reshape(NTILES, P, T, 2, 2, 2, 2, 8)
            .transpose(0, 1, 3, 4, 5, 6, 2, 7)
            .reshape(NTILES, P, FREE)
        )
        tvk = (
            tval[k * rpc : (k + 1) * rpc]
            .reshape(NGROUPS, GK, P, T)
            .transpose(0, 2, 1, 3)
            .reshape(NGROUPS, P, GW)
        )
        in_maps.append(
            {
                "y": np.ascontiguousarray(yk),
                "tv": np.ascontiguousarray(tvk),
                "thr": thr_flat,
            }
        )
    return in_maps


def finalize(gstats_list, n=N):
    """Combine per-core cumulative [3, 11] stats into the ECE scalar."""
    G = np.zeros((3, NB + 1), dtype=np.float64)
    for gs in gstats_list:
        G += gs.astype(np.float64)
    per = G[:, 0:NB] - G[:, 1 : NB + 1]
    counts, sum_conf, sum_acc = per[0], per[1], per[2]
    safe = np.maximum(counts, 1.0)
    avg_conf = sum_conf / safe
    avg_acc = sum_acc / safe
    prop = counts / float(n)
    ece = np.where(counts > 0, np.abs(avg_conf - avg_acc) * prop, 0.0).sum()
    return np.array(ece, dtype=np.float32)


LAST_RESULTS = None  # BassKernelResults of the most recent kernel() call


def kernel(logits, targets):
    global LAST_RESULTS
    from concourse.bass_utils import run_bass_kernel_spmd

    key = tuple(KNOBS["gp_tiles"])
    if key not in _CACHE:
        _CACHE[key] = build(KNOBS["gp_tiles"])
    nc = _CACHE[key]

    in_maps = prep_inputs(logits, targets)
    res = run_bass_kernel_spmd(nc, in_maps, core_ids=list(range(NCORES)))
    LAST_RESULTS = res
    return finalize([r["gstats"] for r in res.results])


# revision 7
# speedup vs baseline: 1.2425x; 1.1526x over previous
"""Trainium2 Bass kernel for expected-calibration-error (ECE) over [N,C] logits.

Contract: kernel(logits, targets) -> np.float32 scalar (shape ()), matching

    probs = softmax(logits); conf = max(probs); pred = argmax(probs)
    acc = (pred == targets); bins of width 1/10 over (k/10, (k+1)/10]
    ECE = sum_k |avg_conf_k - avg_acc_k| * count_k / N

Strategy (data-parallel over 8 NeuronCores, rows sharded, bf16 on device):
  * Host casts logits to bf16 (halves HBM traffic; 2.6e-3 final rel err vs
    the 2e-2 gate) and gathers tval[i] = y16[i, targets[i]]. Since tval is
    an element of row i, acc == (tval == rowmax) reproduces argmax accuracy
    up to bf16 ties.
  * Rows live on partitions; each row's 128 classes are stored in host
    bit-reversed block order [b6,b5,b4,b3,t,c_low3] so every pairwise fold
    level of a reduction tree combines two CONTIGUOUS halves. DVE
    tensor_tensor measures 2 elem/cycle/lane on bf16 (fp16 add is
    emulated ~6x slower; tensor_reduce is 1 elem/cycle) — so per-row max
    and sum-of-exp run as 4 contiguous bf16 folds + one width-8 reduce.
  * exp on the ACT engine (bf16 in/out). For selected tiles the whole sum
    chain runs on GpSimd in fp16 (the Pool engine only implements add, and
    only fp16 is fast there) to offload the DVE.
  * conf = exp(max) * recip(sumexp); cumulative bin masks g[k] = conf>k/10
    as [P, 11, gw] bf16; per-128-row-block matmul(lhsT=[1|conf|acc], rhs=g)
    accumulates cumulative [3,11] stats in PSUM across the whole shard.
  * Host sums the 8 [3,11] outputs, differences adjacent cumulative
    columns, applies the final ECE formula.
"""

import numpy as np

# Problem constants (hardcoded per harness contract).
N = 262144
C = 128
P = 128          # SBUF partitions
NB = 10          # calibration bins
NCORES = 8
T = 32           # rows per partition per tile
GK = 4           # tiles per small-op group
ROWS_PER_CORE = N // NCORES          # 32768
NTILES = ROWS_PER_CORE // (P * T)    # 8
NGROUPS = NTILES // GK
GW = GK * T                          # group width (stat columns)
FREE = T * C                         # 4096 elems per partition per tile

_CACHE = {}

KNOBS = dict(
    # GpSimd chains measured net-negative: the Q7 fp16 software loop starves
    # DVE SBUF access (concurrent DVE folds ran 3-20x slower). Keep GpSimd idle.
    gp_tiles=(),
)


def build(gp_tiles=(1, 4, 6)):
    import concourse.bacc as bacc
    import concourse.tile as tile
    from concourse import mybir

    f16 = mybir.dt.float16
    bf16 = mybir.dt.bfloat16
    f32 = mybir.dt.float32
    Alu = mybir.AluOpType
    Act = mybir.ActivationFunctionType
    X = mybir.AxisListType.X

    nc = bacc.Bacc(trn_type="TRN2")

    y_d = nc.dram_tensor("y", [NTILES, P, FREE], bf16, kind="ExternalInput")
    tv_d = nc.dram_tensor("tv", [NGROUPS, P, GW], bf16, kind="ExternalInput")
    thr_d = nc.dram_tensor("thr", [1, (NB + 1) * GW], bf16, kind="ExternalInput")
    out_d = nc.dram_tensor("gstats", [3, NB + 1], f32, kind="ExternalOutput")

    with nc.allow_low_precision("bf16 ECE pipeline; 2.6e-3 final rel err"):
        with tile.TileContext(nc) as tc:
            with (
                tc.tile_pool(name="io", bufs=3) as io_pool,
                tc.tile_pool(name="eb", bufs=2) as eb_pool,
                tc.tile_pool(name="ef", bufs=2) as ef_pool,
                tc.tile_pool(name="fm", bufs=2) as fm_pool,
                tc.tile_pool(name="fs", bufs=2) as fs_pool,
                tc.tile_pool(name="fg", bufs=2) as fg_pool,
                tc.tile_pool(name="grp", bufs=2) as grp_pool,
                tc.tile_pool(name="single", bufs=1) as single,
                tc.tile_pool(name="psum", bufs=1, space="PSUM") as psum_pool,
            ):
                thrg = single.tile([P, (NB + 1) * GW], bf16)
                nc.sync.dma_start(out=thrg[:], in_=thr_d[:].partition_broadcast(P))
                thrg3 = thrg[:].rearrange("p (a b) -> p a b", b=GW)

                pstats = psum_pool.tile([3, NB + 1], f32)

                def tree(eng, src, out_col, op, pool, dt, tag):
                    """src [P, 4096] (bit-reversed layout) -> out_col [P, T].
                    Four contiguous pairwise folds + one width-8 reduce.
                    Folds run on `eng`; the final reduce always on DVE."""
                    cur, w = src, FREE
                    while w > T * 8:
                        h = w // 2
                        nxt = pool.tile([P, h], dt, name=f"{tag}{h}")
                        eng.tensor_tensor(
                            out=nxt[:], in0=cur[:, 0:h], in1=cur[:, h:w], op=op
                        )
                        cur, w = nxt[:], h
                    nc.vector.tensor_reduce(
                        out=out_col, in_=cur.rearrange("p (t c) -> p t c", c=8),
                        axis=X, op=op,
                    )

                for grp in range(NGROUPS):
                    my_g = grp_pool.tile([P, GW], bf16)
                    s_g = grp_pool.tile([P, GW], bf16)
                    tv_g = grp_pool.tile([P, GW], bf16)
                    nc.sync.dma_start(out=tv_g[:], in_=tv_d[grp])

                    for ti in range(GK):
                        t = grp * GK + ti
                        o0, o1 = ti * T, (ti + 1) * T

                        y_t = io_pool.tile([P, FREE], bf16)
                        nc.sync.dma_start(out=y_t[:], in_=y_d[t])

                        # per-row max of y (DVE folds; Pool has no max op)
                        tree(nc.vector, y_t[:], my_g[:, o0:o1], Alu.max,
                             fm_pool, bf16, "m")

                        # per-row sum of exp(y)
                        if t in gp_tiles:
                            E = ef_pool.tile([P, FREE], f16)
                            nc.scalar.activation(out=E[:], in_=y_t[:], func=Act.Exp)
                            tree(nc.gpsimd, E[:], s_g[:, o0:o1], Alu.add,
                                 fg_pool, f16, "g")
                        else:
                            E = eb_pool.tile([P, FREE], bf16)
                            nc.scalar.activation(out=E[:], in_=y_t[:], func=Act.Exp)
                            tree(nc.vector, E[:], s_g[:, o0:o1], Alu.add,
                                 fs_pool, bf16, "s")

                    # --- batched small per-row ops over the whole group ---
                    maxE = grp_pool.tile([P, GW], bf16)
                    nc.scalar.activation(out=maxE[:], in_=my_g[:], func=Act.Exp)
                    rs = grp_pool.tile([P, GW], bf16)
                    nc.vector.reciprocal(out=rs[:], in_=s_g[:])

                    rhs3 = grp_pool.tile([P, 3, GW], bf16)
                    nc.vector.memset(rhs3[:, 0, :], 1.0)
                    nc.vector.tensor_tensor(
                        out=rhs3[:, 1, :], in0=maxE[:], in1=rs[:], op=Alu.mult
                    )
                    # tval is an element of the row, so tval == max <=> argmax hit
                    nc.vector.tensor_tensor(
                        out=rhs3[:, 2, :], in0=tv_g[:], in1=my_g[:], op=Alu.is_equal
                    )

                    g = grp_pool.tile([P, NB + 1, GW], bf16)
                    conf_b = rhs3[:, 1, :].unsqueeze(1).broadcast_to([P, NB + 1, GW])
                    nc.vector.tensor_tensor(
                        out=g[:], in0=conf_b, in1=thrg3, op=Alu.is_gt
                    )

                    for j in range(GW):
                        nc.tensor.matmul(
                            pstats[:],
                            rhs3[:, :, j],
                            g[:, :, j],
                            start=(grp == 0 and j == 0),
                            stop=(grp == NGROUPS - 1 and j == GW - 1),
                            skip_group_check=True,
                        )

                stats_sb = single.tile([3, NB + 1], f32)
                nc.vector.tensor_copy(out=stats_sb[:], in_=pstats[:])
                nc.sync.dma_start(out=out_d[:], in_=stats_sb[:])

    nc.compile()
    return nc


def prep_inputs(logits, targets):
    """Cast + shard + fold-layout host inputs. Returns per-core in_maps."""
    import ml_dtypes

    bf16 = ml_dtypes.bfloat16
    l = np.asarray(logits, dtype=np.float32)
    tg = np.asarray(targets).astype(np.int64)
    n = l.shape[0]

    y16 = l.astype(bf16)
    tval = y16[np.arange(n), tg]

    thrv = (np.arange(NB + 1, dtype=np.float32) / NB).astype(bf16)
    thrv[NB] = bf16(3e38)  # bin mask 10 must stay empty even if conf rounds >1
    thr_flat = np.ascontiguousarray(np.repeat(thrv, GW).reshape(1, (NB + 1) * GW))

    rpc = n // NCORES
    in_maps = []
    for k in range(NCORES):
        yk = y16[k * rpc : (k + 1) * rpc].reshape(NTILES, P, T, C)
        # bit-reversed block layout: [t, c=(b6 b5 b4 b3 c3)] -> [b6 b5 b4 b3 t c3]
        yk = (
            yk.reshape(NTILES, P, T, 2, 2, 2, 2, 8)
            .transpose(0, 1, 3, 4, 5, 6, 2, 7)
            .reshape(NTILES, P, FREE)
        )
        tvk = (
            tval[k * rpc : (k + 1) * rpc]
            .reshape(NGROUPS, GK, P, T)
            .transpose(0, 2, 1, 3)
            .reshape(NGROUPS, P, GW)
        )
        in_maps.append(
            {
                "y": np.ascontiguousarray(yk),
                "tv": np.ascontiguousarray(tvk),
                "thr": thr_flat,
            }
        )
    return in_maps


def finalize(gstats_list, n=N):
    """Combine per-core cumulative [3, 11] stats into the ECE scalar."""
    G = np.zeros((3, NB + 1), dtype=np.float64)
    for gs in gstats_list:
        G += gs.astype(np.float64)
    per = G[:, 0:NB] - G[:, 1 : NB + 1]
    counts, sum_conf, sum_acc = per[0], per[1], per[2]
    safe = np.maximum(counts, 1.0)
    avg_conf = sum_conf / safe
    avg_acc = sum_acc / safe
    prop = counts / float(n)
    ece = np.where(counts > 0, np.abs(avg_conf - avg_acc) * prop, 0.0).sum()
    return np.array(ece, dtype=np.float32)


LAST_RESULTS = None  # BassKernelResults of the most recent kernel() call


def kernel(logits, targets):
    global LAST_RESULTS
    from concourse.bass_utils import run_bass_kernel_spmd

    key = tuple(KNOBS["gp_tiles"])
    if key not in _CACHE:
        _CACHE[key] = build(KNOBS["gp_tiles"])
    nc = _CACHE[key]

    in_maps = prep_inputs(logits, targets)
    res = run_bass_kernel_spmd(nc, in_maps, core_ids=list(range(NCORES)))
    LAST_RESULTS = res
    return finalize([r["gstats"] for r in res.results])
